# revision 15
# baseline (speedup 1.0000x reference)
"""ColorContrastLoss Trainium2 kernel.

Strategy (data-parallel over B across 8 cores, one batch per core):

The loss depends on pred_masks only through the per-mask color feature
raw[n, c] = sum_hw pred_masks[n, hw] * images[c, hw]  (the area division in
the reference cancels under the subsequent L2 normalization, and
target_masks is unused by the reference entirely).  That contraction over
HW = 147456 is the only heavy work (~19 MB of mask data per core, which
pins the kernel at the ~358 GB/s per-core HBM roofline) and is done on the
TensorEngine without transposing the big tensor:

  - Tiles are loaded "chunk-per-partition": tile[p, n, f] = mask[n, p*F + f]
    with F = HW/128 = 1152, so every DMA descriptor is a contiguous run of
    >= 512 B (line rate).  The mask chunk DMAs are the first instructions
    issued on the sync (HWDGE) queue; the image load rides the scalar
    (HWDGE) queue concurrently, and no other DRAM constants exist (the eye
    pattern is generated on-device with iota), so HBM stays saturated from
    the first microsecond.
  - Chunks within a group shrink toward the end (12, 12, 8, 4 j-windows)
    so the final chunk's matmul burst exposes only ~1.7 us after the last
    HBM byte lands.
  - For residue-window j (32 wide), a matmul with stationary
    img_t[:, j, :, :]  (M = 3*32 = 96 columns, m=(c,r)) and moving
    mask tile slice (N = 16*32 = 512 columns, n=(n16,fr)) produces
    out[(c,r), (n16,fr)] = sum_p img[c, p*F+j*32+r] * mask[n, p*F+j*32+fr].
    Only fr == r entries are wanted; accumulating over all j in PSUM and
    then masking with an eye pattern + free-dim reduce yields exactly
    sum_hw mask[n, hw]*img[c, hw] split by r.
  - The per-core result shipped back is just the [96, 32] masked-reduced
    tile (12 KB).  The tiny epilogue (sum over r, L2-normalize, 32x32
    cosine similarity, relu margin, valid-pair masking, reduction, and the
    cross-core all-reduce of the pair-weighted sums) runs on the host in
    float64, which is exactly the all-reduce the sharding hint describes.
"""

import os
import sys

import numpy as np

for _p in ("/opt/trn_rl_repo", "/root/.axon_site/_ro/trn_rl_repo"):
    if os.path.isdir(_p) and _p not in sys.path:
        sys.path.append(_p)

TEMPERATURE = 0.07
MARGIN = 0.5
WEIGHT = 1.0

B, N, C, H, W = 8, 32, 3, 384, 384
HW = H * W            # 147456
P = 128               # SBUF partitions
F = HW // P           # 1152 elements per partition-chunk
RCH = 32              # residue window width (stationary img columns per j)
NJ = F // RCH         # 36 accumulation steps
GN = 16               # masks per group (moving N = GN*RCH = 512 per matmul)
NG = N // GN          # 2 groups
# f-range chunking per group: HBM line rate (~390-400 GB/s measured) wants
# descriptor runs >= 2 KB (2304 B = 18 j-windows); the taper (12, 6) keeps
# the trailing matmul burst after the last HBM byte short.  Group 0's
# tapered chunks ride the scalar HWDGE queue (behind the image) so their
# small-descriptor rate deficit overlaps the sync queue's full-rate chunks
# and the two queues together hold ~405-415 GB/s.
JQS = (18, 12, 6)      # j-windows per chunk (sums to NJ)
M = C * RCH           # 96 result rows (c, r)
NCORES = 8


def _kernel_body(ctx, tc, mask, img, out):
    from concourse import mybir

    nc = tc.nc
    f32 = mybir.dt.float32
    f32r = mybir.dt.float32r
    i32 = mybir.dt.int32
    ALU = mybir.AluOpType
    AX = mybir.AxisListType

    consts = ctx.enter_context(tc.tile_pool(name="consts", bufs=1))
    mpool = ctx.enter_context(tc.tile_pool(name="maskp", bufs=2))
    epool = ctx.enter_context(tc.tile_pool(name="extr", bufs=2))
    spool = ctx.enter_context(tc.tile_pool(name="small", bufs=1))
    psum = ctx.enter_context(tc.tile_pool(name="psum", bufs=2, space="PSUM"))

    # --- mask chunk loads first: keep both HWDGE queues (and HBM) busy
    # from the first possible cycle ---
    f0s = []
    f0 = 0
    for jq in JQS:
        f0s.append(f0)
        f0 += jq * RCH
    mtiles = [[None] * len(JQS) for _ in range(NG)]

    def load_chunk(engine, g, q):
        fq = JQS[q] * RCH
        t = mpool.tile(
            [P, GN, fq], f32r, tag=f"mask{g}_{q}", bufs=1, name=f"mask_g{g}q{q}"
        )
        src_g = mask[g * GN : (g + 1) * GN, :].rearrange("n (p f) -> p n f", p=P)
        engine.dma_start(
            out=t[:], in_=src_g[:, :, f0s[q] : f0s[q] + fq].bitcast(f32r)
        )
        mtiles[g][q] = t

    load_chunk(nc.sync, 0, 0)

    # --- image + group-0 taper chunks on the scalar HWDGE queue (parallel
    # FIFO); the image is then reshuffled on DVE into j-major [p, j, c, r]
    # so each matmul's stationary slice [:, j, :, :] merges to a single
    # free dimension ---
    img_raw = consts.tile([P, C, F], f32)
    nc.scalar.dma_start(
        out=img_raw[:], in_=img.rearrange("c (p f) -> p c f", p=P)
    )
    load_chunk(nc.scalar, 0, 1)
    load_chunk(nc.scalar, 0, 2)
    load_chunk(nc.sync, 1, 0)
    load_chunk(nc.sync, 1, 1)
    load_chunk(nc.sync, 1, 2)
    img_t = consts.tile([P, NJ, C, RCH], f32r)
    nc.vector.tensor_copy(
        out=img_t[:],
        in_=img_raw[:].rearrange("p c (j r) -> p j c r", r=RCH).bitcast(f32r),
    )

    # --- eye pattern generated on-device: eyepat[(c,r), (n,fr)] = (fr == r).
    # iota value = fr - p; (fr - p) & 31 == 0  <=>  fr == p mod 32. ---
    ii = consts.tile([M, GN, RCH], i32)
    nc.gpsimd.iota(
        ii[:], pattern=[[0, GN], [1, RCH]], base=0, channel_multiplier=-1
    )
    ib = consts.tile([M, GN, RCH], i32)
    nc.vector.tensor_scalar(
        out=ib[:], in0=ii[:], scalar1=31, scalar2=None, op0=ALU.bitwise_and
    )
    eyepat = consts.tile([M, GN, RCH], f32)
    nc.vector.tensor_scalar(
        out=eyepat[:], in0=ib[:], scalar1=0, scalar2=None, op0=ALU.is_equal
    )

    collected = spool.tile([M, N], f32)

    # --- main contraction ---
    for g in range(NG):
        acc = psum.tile([P, GN, RCH], f32, tag="acc")
        j = 0
        for q, jq in enumerate(JQS):
            for t in range(jq):
                nc.tensor.matmul(
                    acc[0:M],
                    lhsT=img_t[:, j, :, :],
                    rhs=mtiles[g][q][:, :, t * RCH : (t + 1) * RCH],
                    start=(j == 0),
                    stop=(j == NJ - 1),
                )
                j += 1
        masked = epool.tile([M, GN, RCH], f32, tag="masked")
        nc.vector.tensor_mul(masked[:], acc[0:M], eyepat[:])
        nc.vector.tensor_reduce(
            out=collected[:, g * GN : (g + 1) * GN],
            in_=masked[:],
            axis=AX.X,
            op=ALU.add,
        )

    nc.sync.dma_start(out=out, in_=collected[:])


def _build_bass():
    import concourse.bacc as bacc
    import concourse.tile as tile
    from concourse import mybir
    from concourse._compat import with_exitstack

    nc = bacc.Bacc(
        "TRN2", target_bir_lowering=False, debug=False, num_devices=NCORES
    )
    f32 = mybir.dt.float32
    mask = nc.dram_tensor("mask", [N, HW], f32, kind="ExternalInput").ap()
    img = nc.dram_tensor("img", [C, HW], f32, kind="ExternalInput").ap()
    out = nc.dram_tensor("out", [M, N], f32, kind="ExternalOutput").ap()

    body = with_exitstack(_kernel_body)
    with tile.TileContext(nc) as tc:
        body(tc, mask, img, out)
    nc.compile()
    return nc


_NC_CACHE = None


def _get_nc():
    global _NC_CACHE
    if _NC_CACHE is None:
        _NC_CACHE = _build_bass()
    return _NC_CACHE


def _run_on_device(pred, imgs, trace=False, tmpdir=None):
    from concourse.bass_utils import run_bass_kernel_spmd

    nc = _get_nc()
    in_maps = []
    for b in range(NCORES):
        in_maps.append(
            {
                "mask": np.ascontiguousarray(pred[b].reshape(N, HW)),
                "img": np.ascontiguousarray(imgs[b].reshape(C, HW)),
            }
        )
    return run_bass_kernel_spmd(
        nc, in_maps, core_ids=list(range(NCORES)), trace=trace, tmpdir=tmpdir
    )


def kernel(pred_masks, target_masks, images, valid_mask, _trace=False, _tmpdir=None):
    pred = np.asarray(pred_masks, dtype=np.float32)
    imgs = np.asarray(images, dtype=np.float32)
    valid = np.asarray(valid_mask, dtype=np.float64)

    res = _run_on_device(pred, imgs, trace=_trace, tmpdir=_tmpdir)

    # Host epilogue (the scalar all-reduce across cores): collected[(c,r), n]
    # -> raw colors -> cosine similarities -> margin/valid-masked pair sum.
    eye = 1.0 - np.eye(N)
    csum = 0.0
    num_pairs = 0.0
    for b in range(NCORES):
        col = np.asarray(res.results[b]["out"], dtype=np.float64)  # [M, N]
        raw = col.reshape(C, RCH, N).sum(axis=1).T  # [N, C]
        nrm = np.maximum(np.linalg.norm(raw, axis=1, keepdims=True), 1e-12)
        z = raw / nrm
        sim = (z @ z.T) / TEMPERATURE
        inst = eye * np.outer(valid[b], valid[b])
        csum += (np.maximum(sim - MARGIN, 0.0) * inst).sum()
        num_pairs += inst.sum()
    loss = np.float32(csum / (num_pairs + 1e-6) * WEIGHT)
    if _trace:
        return loss, res
    return loss


# revision 20
# speedup vs baseline: 1.0491x; 1.0491x over previous
"""ColorContrastLoss Trainium2 kernel.

Strategy (data-parallel over B across 8 cores, one batch per core):

The loss depends on pred_masks only through the per-mask color feature
raw[n, c] = sum_hw pred_masks[n, hw] * images[c, hw]  (the area division in
the reference cancels under the subsequent L2 normalization, and
target_masks is unused by the reference entirely).  That contraction over
HW = 147456 is the only heavy work (~19 MB of mask data per core, which
pins the kernel at the ~358 GB/s per-core HBM roofline) and is done on the
TensorEngine without transposing the big tensor:

  - Tiles are loaded "chunk-per-partition": tile[p, n, f] = mask[n, p*F + f]
    with F = HW/128 = 1152, so every DMA descriptor is a contiguous run of
    >= 512 B (line rate).  The mask chunk DMAs are the first instructions
    issued on the sync (HWDGE) queue; the image load rides the scalar
    (HWDGE) queue concurrently, and no other DRAM constants exist (the eye
    pattern is generated on-device with iota), so HBM stays saturated from
    the first microsecond.
  - Chunks within a group shrink toward the end (12, 12, 8, 4 j-windows)
    so the final chunk's matmul burst exposes only ~1.7 us after the last
    HBM byte lands.
  - For residue-window j (32 wide), a matmul with stationary
    img_t[:, j, :, :]  (M = 3*32 = 96 columns, m=(c,r)) and moving
    mask tile slice (N = 16*32 = 512 columns, n=(n16,fr)) produces
    out[(c,r), (n16,fr)] = sum_p img[c, p*F+j*32+r] * mask[n, p*F+j*32+fr].
    Only fr == r entries are wanted; accumulating over all j in PSUM and
    then masking with an eye pattern + free-dim reduce yields exactly
    sum_hw mask[n, hw]*img[c, hw] split by r.
  - The per-core result shipped back is just the [96, 32] masked-reduced
    tile (12 KB).  The tiny epilogue (sum over r, L2-normalize, 32x32
    cosine similarity, relu margin, valid-pair masking, reduction, and the
    cross-core all-reduce of the pair-weighted sums) runs on the host in
    float64, which is exactly the all-reduce the sharding hint describes.
"""

import os
import sys

import numpy as np

for _p in ("/opt/trn_rl_repo", "/root/.axon_site/_ro/trn_rl_repo"):
    if os.path.isdir(_p) and _p not in sys.path:
        sys.path.append(_p)

TEMPERATURE = 0.07
MARGIN = 0.5
WEIGHT = 1.0

B, N, C, H, W = 8, 32, 3, 384, 384
HW = H * W            # 147456
P = 128               # SBUF partitions
F = HW // P           # 1152 elements per partition-chunk
RCH = 32              # residue window width (stationary img columns per j)
NJ = F // RCH         # 36 accumulation steps
GN = 16               # masks per group (moving N = GN*RCH = 512 per matmul)
NG = N // GN          # 2 groups
# f-range chunking per group: HBM line rate (~390-400 GB/s measured) wants
# descriptor runs >= 2 KB (2304 B = 18 j-windows); the taper (14, 4) keeps
# the trailing matmul burst after the last HBM byte short (4 matmuls)
# without paying the small-descriptor rate penalty on much data.  All mask
# chunks ride the sync HWDGE queue in group order (the scalar queue only
# carries the image up front: its arbitration share decays when busy
# alongside the sync queue, so mid-stream chunks must not live there).
JQS = (18, 14, 4)      # j-windows per chunk (sums to NJ)
M = C * RCH           # 96 result rows (c, r)
NCORES = 8


def _kernel_body(ctx, tc, mask, img, out):
    from concourse import mybir

    nc = tc.nc
    f32 = mybir.dt.float32
    f32r = mybir.dt.float32r
    i32 = mybir.dt.int32
    ALU = mybir.AluOpType
    AX = mybir.AxisListType

    consts = ctx.enter_context(tc.tile_pool(name="consts", bufs=1))
    mpool = ctx.enter_context(tc.tile_pool(name="maskp", bufs=2))
    epool = ctx.enter_context(tc.tile_pool(name="extr", bufs=2))
    spool = ctx.enter_context(tc.tile_pool(name="small", bufs=1))
    psum = ctx.enter_context(tc.tile_pool(name="psum", bufs=2, space="PSUM"))

    # --- mask chunk loads first: keep both HWDGE queues (and HBM) busy
    # from the first possible cycle ---
    f0s = []
    f0 = 0
    for jq in JQS:
        f0s.append(f0)
        f0 += jq * RCH
    mtiles = [[None] * len(JQS) for _ in range(NG)]

    def load_chunk(engine, g, q):
        fq = JQS[q] * RCH
        t = mpool.tile(
            [P, GN, fq], f32r, tag=f"mask{q}", bufs=1, name=f"mask_g{g}q{q}"
        )
        src_g = mask[g * GN : (g + 1) * GN, :].rearrange("n (p f) -> p n f", p=P)
        engine.dma_start(
            out=t[:], in_=src_g[:, :, f0s[q] : f0s[q] + fq].bitcast(f32r)
        )
        mtiles[g][q] = t

    load_chunk(nc.sync, 0, 0)

    # --- image on the scalar HWDGE queue (parallel FIFO), then reshuffled
    # on DVE into j-major [p, j, c, r] so each matmul's stationary slice
    # [:, j, :, :] merges to a single free dimension ---
    img_raw = consts.tile([P, C, F], f32)
    nc.scalar.dma_start(
        out=img_raw[:], in_=img.rearrange("c (p f) -> p c f", p=P)
    )
    load_chunk(nc.sync, 0, 1)
    load_chunk(nc.sync, 0, 2)
    load_chunk(nc.sync, 1, 0)
    load_chunk(nc.sync, 1, 1)
    load_chunk(nc.sync, 1, 2)
    img_t = consts.tile([P, NJ, C, RCH], f32r)
    nc.vector.tensor_copy(
        out=img_t[:],
        in_=img_raw[:].rearrange("p c (j r) -> p j c r", r=RCH).bitcast(f32r),
    )

    # --- eye pattern generated on-device: eyepat[(c,r), (n,fr)] = (fr == r).
    # iota value = fr - p; (fr - p) & 31 == 0  <=>  fr == p mod 32. ---
    ii = consts.tile([M, GN, RCH], i32)
    nc.gpsimd.iota(
        ii[:], pattern=[[0, GN], [1, RCH]], base=0, channel_multiplier=-1
    )
    ib = consts.tile([M, GN, RCH], i32)
    nc.vector.tensor_scalar(
        out=ib[:], in0=ii[:], scalar1=31, scalar2=None, op0=ALU.bitwise_and
    )
    eyepat = consts.tile([M, GN, RCH], f32)
    nc.vector.tensor_scalar(
        out=eyepat[:], in0=ib[:], scalar1=0, scalar2=None, op0=ALU.is_equal
    )

    collected = spool.tile([M, N], f32)

    # --- main contraction ---
    for g in range(NG):
        acc = psum.tile([P, GN, RCH], f32, tag="acc")
        j = 0
        for q, jq in enumerate(JQS):
            for t in range(jq):
                nc.tensor.matmul(
                    acc[0:M],
                    lhsT=img_t[:, j, :, :],
                    rhs=mtiles[g][q][:, :, t * RCH : (t + 1) * RCH],
                    start=(j == 0),
                    stop=(j == NJ - 1),
                )
                j += 1
        # eye-mask + reduce over fr (DVE only: GpSimd cannot access PSUM)
        masked = epool.tile([M, GN, RCH], f32, tag="masked")
        nc.vector.tensor_mul(masked[:], acc[0:M], eyepat[:])
        nc.vector.tensor_reduce(
            out=collected[:, g * GN : (g + 1) * GN],
            in_=masked[:],
            axis=AX.X,
            op=ALU.add,
        )

    nc.scalar.dma_start(out=out, in_=collected[:])


def _build_bass():
    import concourse.bacc as bacc
    import concourse.tile as tile
    from concourse import mybir
    from concourse._compat import with_exitstack

    nc = bacc.Bacc(
        "TRN2", target_bir_lowering=False, debug=False, num_devices=NCORES
    )
    f32 = mybir.dt.float32
    mask = nc.dram_tensor("mask", [N, HW], f32, kind="ExternalInput").ap()
    img = nc.dram_tensor("img", [C, HW], f32, kind="ExternalInput").ap()
    out = nc.dram_tensor("out", [M, N], f32, kind="ExternalOutput").ap()

    body = with_exitstack(_kernel_body)
    with tile.TileContext(nc) as tc:
        body(tc, mask, img, out)
    nc.compile()
    return nc


_NC_CACHE = None


def _get_nc():
    global _NC_CACHE
    if _NC_CACHE is None:
        _NC_CACHE = _build_bass()
    return _NC_CACHE


def _run_on_device(pred, imgs, trace=False, tmpdir=None):
    from concourse.bass_utils import run_bass_kernel_spmd

    nc = _get_nc()
    in_maps = []
    for b in range(NCORES):
        in_maps.append(
            {
                "mask": np.ascontiguousarray(pred[b].reshape(N, HW)),
                "img": np.ascontiguousarray(imgs[b].reshape(C, HW)),
            }
        )
    return run_bass_kernel_spmd(
        nc, in_maps, core_ids=list(range(NCORES)), trace=trace, tmpdir=tmpdir
    )


def kernel(pred_masks, target_masks, images, valid_mask, _trace=False, _tmpdir=None):
    pred = np.asarray(pred_masks, dtype=np.float32)
    imgs = np.asarray(images, dtype=np.float32)
    valid = np.asarray(valid_mask, dtype=np.float64)

    res = _run_on_device(pred, imgs, trace=_trace, tmpdir=_tmpdir)

    # Host epilogue (the scalar all-reduce across cores): collected[(c,r), n]
    # -> raw colors -> cosine similarities -> margin/valid-masked pair sum.
    eye = 1.0 - np.eye(N)
    csum = 0.0
    num_pairs = 0.0
    for b in range(NCORES):
        col = np.asarray(res.results[b]["out"], dtype=np.float64)  # [M, N]
        raw = col.reshape(C, RCH, N).sum(axis=1).T  # [N, C]
        nrm = np.maximum(np.linalg.norm(raw, axis=1, keepdims=True), 1e-12)
        z = raw / nrm
        sim = (z @ z.T) / TEMPERATURE
        inst = eye * np.outer(valid[b], valid[b])
        csum += (np.maximum(sim - MARGIN, 0.0) * inst).sum()
        num_pairs += inst.sum()
    loss = np.float32(csum / (num_pairs + 1e-6) * WEIGHT)
    if _trace:
        return loss, res
    return loss
